# revision 3
# baseline (speedup 1.0000x reference)
"""Trainium2 Bass kernel for nn_A2SSM (depthwise 5x5 conv + SiLU + selective
scan, d_state=1), data-parallel over batch across 8 NeuronCores.

Self-contained: host code shards inputs (2 batches/core), pre-permutes weights
into PE-friendly layouts, runs one SPMD Bass program on cores 0-7, gathers.

Device dataflow per core (384 lanes = 2 batches x 192 channels, L=4096),
processed in 4 L-chunks of 1024 (16 image rows):
  - depthwise conv: 12 groups of 32 lanes on independent 32x32 PE tiles
    (row-group = x location, col-group = PSUM lane home), 25 taps accumulated
    per PSUM bank with border handling via clipped access patterns
  - SiLU (ACT, bias fused) -> u [bf16]
  - x_proj / delta projections on the spare PE row-group (partitions 96-127)
    fed by on-chip DMA re-layouts (PSUM accumulation never crosses row groups:
    hardware faults otherwise)
  - A_logs == 0 (always true for this model) => decay a = sigmoid(-z) and
    -delta = ln(a), both single ACT passes; sign carried through the scan and
    fixed in the final scalar_tensor_tensor (u*Ds - (-h*Cs))
  - selective scan via DVE tensor_tensor_scan, chained across chunks
  - Bs/Cs broadcasts via DRAM-bounce replicate DMAs
Output returned bf16 and widened to f32 on host.
"""
import sys

if "/opt/trn_rl_repo" not in sys.path:
    sys.path.insert(0, "/opt/trn_rl_repo")

import numpy as np
import ml_dtypes

import concourse.bass as bass
import concourse.mybir as mybir
import concourse.tile as tile
from concourse.tile import TileContext
from concourse.vector_clock import ScopedClock

F32 = mybir.dt.float32
BF16 = mybir.dt.bfloat16
AF = mybir.ActivationFunctionType
ALU = mybir.AluOpType

# ---------------------------------------------------------------------------
# Tile/walrus compatibility patches: this neuronxcc build supports at most ONE
# sync-wait per instruction; TileContext parks multiple waits on single
# instructions (tail drain + regular insts). Peel extras onto NoOps.
# ---------------------------------------------------------------------------
_PEEL = [0]


def _patched_add_instruction(self, inst):
    si = getattr(inst, "sync_info", None)
    if (
        si is not None
        and si.on_wait
        and len(si.on_wait) > 1
        and getattr(inst, "engine", None) is not None
        and inst.engine != mybir.EngineType.Unassigned
    ):
        waits = list(si.on_wait)
        for w in waits[:-1]:
            _PEEL[0] += 1
            nop = mybir.InstNoOp(
                name=f"{inst.name}_pw{_PEEL[0]}",
                sync_info=mybir.SyncInfo(on_wait=[w], on_update=[]),
                bass_nofuse=True,
                engine=inst.engine,
            )
            _orig_add_instruction(self, nop)
        si.on_wait = [waits[-1]]
        inst.sync_info = si
    _orig_add_instruction(self, inst)


def _patched_drain_and_barrier(self, tick_clock, wait_clock):
    nc = self.nc
    nop0 = nc.sync.nop(nofuse=True, hint="tail_wait0")
    wait_clock.add_sem_waits(nop0.ins, ScopedClock({None: tick_clock.global_clock}))
    si = nop0.ins.sync_info
    waits = list(si.on_wait) if si is not None and si.on_wait else []
    if len(waits) > 1:
        si.on_wait = [waits[0]]
        for w in waits[1:]:
            nopk = nc.sync.nop(nofuse=True, hint="tail_waitk")
            nopk.ins.sync_info = mybir.SyncInfo(on_wait=[w], on_update=[])
    nc.sync.drain()
    nc.all_engine_barrier()
    assert self.sems is not None
    popped = nc._tile_sem_poison_stack.pop()
    assert popped is self._sem_poison
    nc.clear_and_free_semaphores(list(self.sems.allocated().values()))
    nc.all_engine_barrier()


_orig_add_instruction = tile.TileContext._add_instruction
tile.TileContext._add_instruction = _patched_add_instruction
tile.TileContext._drain_and_barrier = _patched_drain_and_barrier

# ---------------------------------------------------------------------------
# Problem constants (hardcoded per the task spec)
# ---------------------------------------------------------------------------
B, C, H, W = 16, 192, 64, 64
L = H * W
DT_RANK = 12
N_CORES = 8
B_LOC = B // N_CORES          # 2 batches per core
LANES = B_LOC * C             # 384
NCH = 4                       # L-chunks
LC = L // NCH                 # 1024 columns per chunk
RCH = 16                      # image rows per chunk
TAPORD = [12] + [k for k in range(25) if k != 12]  # center tap first


def _chunk_window(c):
    """Input image-row window needed for output rows [16c, 16c+16)."""
    h0 = max(0, 16 * c - 2)
    h1 = min(H, 16 * c + 18)
    return h0, h1


def _build_program():
    nc = bass.Bass(trn_type="TRN2", target_bir_lowering=False, debug=False,
                   num_devices=N_CORES)

    xh_e = nc.dram_tensor("xh", [96, 4, L], BF16, kind="ExternalInput").ap()
    wd_e = nc.dram_tensor("wd", [96, 4, 25, 32], BF16, kind="ExternalInput").ap()
    xpw_e = nc.dram_tensor("xpw", [32, 12, 32], BF16, kind="ExternalInput").ap()
    dtw_e = nc.dram_tensor("dtw", [12, 12, 32], BF16, kind="ExternalInput").ap()
    lp_e = nc.dram_tensor("lp", [128, 3, 3], F32, kind="ExternalInput").ap()
    out_e = nc.dram_tensor("out", [LANES, L], BF16, kind="ExternalOutput").ap()
    scratch = nc.dram_tensor("bscratch", [NCH, 4, LC], BF16).ap()

    with TileContext(nc) as tc:
        with tc.tile_pool(name="consts", bufs=1) as consts, \
             tc.tile_pool(name="xc", bufs=2) as xcp, \
             tc.tile_pool(name="us", bufs=1) as usp, \
             tc.tile_pool(name="work", bufs=1) as wk, \
             tc.tile_pool(name="cps", bufs=4, space="PSUM") as cps, \
             tc.tile_pool(name="xds", bufs=1, space="PSUM") as xds, \
             tc.tile_pool(name="dps", bufs=1, space="PSUM") as dps:

            # ---- constants (loaded once)
            wd_t = consts.tile([96, 4, 25, 32], BF16)
            nc.sync.dma_start(out=wd_t[:, :, :, :], in_=wd_e[:, :, :, :])
            xpw_t = consts.tile([128, 12, 32], BF16)
            nc.sync.dma_start(out=xpw_t[96:128, :, :], in_=xpw_e[:, :, :])
            dtw_t = consts.tile([128, 12, 32], BF16)
            nc.sync.dma_start(out=dtw_t[96:108, :, :], in_=dtw_e[:, :, :])
            lp_t = consts.tile([128, 3, 3], F32)
            nc.sync.dma_start(out=lp_t[:, :, :], in_=lp_e[:, :, :])

            h_prev = [None, None, None]
            for c in range(NCH):
                h0in, h1in = _chunk_window(c)
                rows_in = h1in - h0in

                # ---- x chunk (issued from ACT's HWDGE queue)
                x_c = xcp.tile([96, 4, 20, 64], BF16, tag="xc", name=f"xc_{c}")
                nc.scalar.dma_start(
                    out=x_c[:, :, 0:rows_in, :],
                    in_=xh_e[:, :, h0in * 64:h1in * 64].rearrange(
                        "p d (r w) -> p d r w", w=64),
                )

                # ---- depthwise conv: 12 groups on (row=t, col=d) PE tiles
                cp = {}
                for t in range(3):
                    for bank in range(2):
                        cp[(t, bank)] = cps.tile([128, 8, 64], F32, tag="cp", name=f"cp_{c}_{t}_{bank}")
                for bank in range(2):
                    H0 = 16 * c + 8 * bank
                    for ki, k in enumerate(TAPORD):
                        dy, dx = k // 5 - 2, k % 5 - 2
                        rlo = max(H0, -dy if dy < 0 else 0)
                        rhi = min(H0 + 8, 64 - (dy if dy > 0 else 0))
                        ws = -dx if dx < 0 else 0
                        we = 64 - (dx if dx > 0 else 0)
                        if rlo >= rhi:
                            continue
                        for t in range(3):
                            for d in range(4):
                                out_ap = cp[(t, bank)][
                                    32 * d:32 * d + 32, rlo - H0:rhi - H0, ws:we]
                                in_ap = x_c[
                                    32 * t:32 * t + 32, d,
                                    rlo + dy - h0in:rhi + dy - h0in,
                                    ws + dx:we + dx]
                                nc.tensor.matmul(
                                    out_ap,
                                    wd_t[32 * t:32 * t + 32, d, k, :],
                                    in_ap,
                                    start=(ki == 0),
                                    stop=(ki == len(TAPORD) - 1),
                                    tile_position=(32 * t, 32 * d),
                                    skip_group_check=True,
                                )

                # ---- SiLU (+conv bias) -> u  [bf16]
                u = []
                for t in range(3):
                    u_t = wk.tile([128, LC], BF16, tag=f"u{t}", name=f"u_{c}_{t}", bufs=2)
                    for bank in range(2):
                        nc.scalar.activation(
                            u_t[:, bank * 512:(bank + 1) * 512].rearrange(
                                "p (r w) -> p r w", w=64),
                            cp[(t, bank)][:, :, :],
                            AF.Silu,
                            bias=lp_t[:, t, 0:1],
                        )
                    u.append(u_t)

                # ---- re-layout u into spare row-group for x_proj (K-stacked)
                us_t = usp.tile([128, 12, LC], BF16, tag="us", name=f"us_{c}")
                for bt in range(2):
                    for q in range(6):
                        lane0 = bt * 192 + 32 * q
                        tu, ru = lane0 // 128, lane0 % 128
                        nc.sync.dma_start(
                            out=us_t[96:128, bt * 6 + q, :],
                            in_=u[tu][ru:ru + 32, :],
                        )

                # ---- x_proj: two accumulation groups on tiles (96,0)/(96,32)
                xd = xds.tile([64, LC], F32, tag="xd", name=f"xd_{c}")
                for half in range(2):
                    for q in range(6):
                        for bt in range(2):
                            nc.tensor.matmul(
                                xd[32 * bt:32 * bt + 32,
                                   half * 512:(half + 1) * 512],
                                xpw_t[96:128, bt * 6 + q, :],
                                us_t[96:128, bt * 6 + q,
                                     half * 512:(half + 1) * 512],
                                start=(q == 0),
                                stop=(q == 5),
                                tile_position=(96, 32 * bt),
                                skip_group_check=True,
                            )

                # ---- x_dbl -> SBUF bf16 (dts rows 0:12/32:44, Bs/Cs 12,13/44,45)
                dts = wk.tile([64, LC], BF16, tag="dts", name=f"dts_{c}", bufs=2)
                nc.scalar.copy(dts[:, :], xd[:, :])

                # Bs/Cs rows -> DRAM scratch (for replicate broadcast)
                nc.sync.dma_start(out=scratch[c, 0:2, :], in_=dts[12:14, :])
                nc.sync.dma_start(out=scratch[c, 2:4, :], in_=dts[44:46, :])

                # dts -> spare row-group for the delta matmul
                dt2 = wk.tile([128, 2, LC], BF16, tag="dt2", name=f"dt2_{c}", bufs=2)
                nc.sync.dma_start(out=dt2[96:108, 0, :], in_=dts[0:12, :])
                nc.sync.dma_start(out=dt2[96:108, 1, :], in_=dts[32:44, :])

                # ---- delta pre-activation z (per lane-tile), then
                #      a = sigmoid(-z), -delta = ln(a)   [A == -1 fast path]
                av, negd = [], []
                for t in range(3):
                    d_ps = dps.tile([128, LC], F32, tag="d", name=f"d_{c}_{t}")
                    for j in range(4):
                        bt = 1 if (128 * t + 32 * j) >= 192 else 0
                        for half in range(2):
                            nc.tensor.matmul(
                                d_ps[32 * j:32 * j + 32,
                                     half * 512:(half + 1) * 512],
                                dtw_t[96:108, 4 * t + j, :],
                                dt2[96:108, bt, half * 512:(half + 1) * 512],
                                start=True, stop=True,
                                tile_position=(96, 32 * j),
                                skip_group_check=True,
                            )
                    av_t = wk.tile([128, LC], BF16, tag=f"av{t}", name=f"av_{c}_{t}", bufs=2)
                    nc.scalar.activation(av_t[:, :], d_ps[:, :], AF.Sigmoid,
                                         scale=-1.0, bias=lp_t[:, t, 1:2])
                    negd_t = wk.tile([128, LC], BF16, tag=f"negd{t}", name=f"negd_{c}_{t}", bufs=2)
                    nc.scalar.activation(negd_t[:, :], av_t[:, :], AF.Ln)
                    av.append(av_t)
                    negd.append(negd_t)

                # ---- Bs/Cs broadcast via DRAM replicate
                bscs = []
                for t in range(3):
                    bc_t = wk.tile([128, 2, LC], BF16, tag=f"bscs{t}", name=f"bscs_{c}_{t}", bufs=2)
                    if t == 0:
                        srcs = [(0, 128, 0)]
                    elif t == 2:
                        srcs = [(0, 128, 2)]
                    else:
                        srcs = [(0, 64, 0), (64, 128, 2)]
                    for p0, p1, row in srcs:
                        s = scratch[c, row:row + 2, :]
                        rep = bass.AP(tensor=s.tensor, offset=s.offset,
                                      ap=[[0, p1 - p0]] + s.ap)
                        nc.gpsimd.dma_start(out=bc_t[p0:p1, :, :], in_=rep)
                    bscs.append(bc_t)

                # ---- DVE: db, b_el, scan, hC, y
                for t in range(3):
                    db = wk.tile([128, LC], BF16, tag="db", name=f"db_{c}_{t}", bufs=3)
                    nc.vector.tensor_mul(db[:, :], negd[t][:, :], u[t][:, :])
                    bel = wk.tile([128, LC], BF16, tag="bel", name=f"bel_{c}_{t}", bufs=3)
                    nc.vector.tensor_mul(bel[:, :], db[:, :], bscs[t][:, 0, :])
                    h_t = wk.tile([128, LC], BF16, tag=f"h{t}", name=f"h_{c}_{t}", bufs=2)
                    init = 0.0 if c == 0 else h_prev[t][:, LC - 1:LC]
                    nc.vector.tensor_tensor_scan(
                        h_t[:, :], av[t][:, :], bel[:, :], init,
                        op0=ALU.mult, op1=ALU.add)
                    h_prev[t] = h_t
                    hc = wk.tile([128, LC], BF16, tag="hc", name=f"hc_{c}_{t}", bufs=3)
                    nc.vector.tensor_mul(hc[:, :], h_t[:, :], bscs[t][:, 1, :])
                    y_t = wk.tile([128, LC], BF16, tag="yb", name=f"y_{c}_{t}", bufs=3)
                    nc.vector.scalar_tensor_tensor(
                        y_t[:, :], u[t][:, :], lp_t[:, t, 2:3], hc[:, :],
                        op0=ALU.mult, op1=ALU.subtract)
                    nc.sync.dma_start(
                        out=out_e[128 * t:128 * t + 128, c * LC:(c + 1) * LC],
                        in_=y_t[:, :])
    return nc


# ---------------------------------------------------------------------------
# Host-side: weight preprocessing, SPMD execution (cached PJRT executable)
# ---------------------------------------------------------------------------
_RUNNER = None


def _lane_channels(t):
    """Channel index for each of the 128 partitions of lane-tile t."""
    lanes = 128 * t + np.arange(128)
    return lanes % C


def _prep_params(conv_w, conv_b, x_proj_w, dt_w, dt_b, A_logs, Ds):
    cw = np.asarray(conv_w, np.float32).reshape(C, 25)
    conv_b = np.asarray(conv_b, np.float32)
    x_proj_w = np.asarray(x_proj_w, np.float32)
    dt_w = np.asarray(dt_w, np.float32)
    dt_b = np.asarray(dt_b, np.float32)
    Ds = np.asarray(Ds, np.float32)

    wd = np.zeros((3, 32, 4, 25, 32), np.float32)
    for t in range(3):
        for d in range(4):
            lanes = 128 * t + 32 * d + np.arange(32)
            wv = cw[lanes % C]  # [32, 25]
            for r in range(32):
                wd[t, r, d, :, r] = wv[r]
    wd = wd.reshape(96, 4, 25, 32).astype(ml_dtypes.bfloat16)

    xpw = np.zeros((32, 12, 32), np.float32)
    for bt in range(2):
        for q in range(6):
            xpw[:, bt * 6 + q, 0:14] = x_proj_w[:, 32 * q:32 * q + 32].T
    xpw = xpw.astype(ml_dtypes.bfloat16)

    dtw = np.zeros((12, 12, 32), np.float32)
    for t in range(3):
        for j in range(4):
            lanes = 128 * t + 32 * j + np.arange(32)
            dtw[:, 4 * t + j, :] = dt_w[lanes % C].T
    dtw = dtw.astype(ml_dtypes.bfloat16)

    lp = np.zeros((128, 3, 3), np.float32)
    for t in range(3):
        ch = _lane_channels(t)
        lp[:, t, 0] = conv_b[ch]
        lp[:, t, 1] = -dt_b[ch]
        lp[:, t, 2] = Ds[ch]
    return wd, xpw, dtw, lp


def _get_runner():
    global _RUNNER
    if _RUNNER is not None:
        return _RUNNER
    import jax
    from jax.sharding import Mesh, PartitionSpec
    from jax.experimental.shard_map import shard_map
    from concourse.bass2jax import (_bass_exec_p, install_neuronx_cc_hook,
                                    partition_id_tensor)

    nc = _build_program()
    install_neuronx_cc_hook()

    partition_name = nc.partition_id_tensor.name if nc.partition_id_tensor else None
    in_names, out_names, out_avals, zero_outs = [], [], [], []
    for alloc in nc.m.functions[0].allocations:
        if not isinstance(alloc, mybir.MemoryLocationSet):
            continue
        name = alloc.memorylocations[0].name
        if alloc.kind == "ExternalInput":
            if name != partition_name:
                in_names.append(name)
        elif alloc.kind == "ExternalOutput":
            out_names.append(name)
            shape = tuple(alloc.tensor_shape)
            dtype = mybir.dt.np(alloc.dtype)
            out_avals.append(jax.core.ShapedArray(shape, dtype))
            zero_outs.append(np.zeros(shape, dtype))
    n_params = len(in_names)
    n_outs = len(out_avals)
    all_in_names = list(in_names) + list(out_names)
    if partition_name is not None:
        all_in_names.append(partition_name)
    donate = tuple(range(n_params, n_params + n_outs))

    def _body(*args):
        operands = list(args)
        if partition_name is not None:
            operands.append(partition_id_tensor())
        outs = _bass_exec_p.bind(
            *operands,
            out_avals=tuple(out_avals),
            in_names=tuple(all_in_names),
            out_names=tuple(out_names),
            lowering_input_output_aliases=(),
            sim_require_finite=True,
            sim_require_nnan=True,
            nc=nc,
        )
        return tuple(outs)

    devices = jax.devices()[:N_CORES]
    mesh = Mesh(np.asarray(devices), ("core",))
    in_specs = (PartitionSpec("core"),) * (n_params + n_outs)
    out_specs = (PartitionSpec("core"),) * n_outs
    sharded = jax.jit(
        shard_map(_body, mesh=mesh, in_specs=in_specs, out_specs=out_specs,
                  check_rep=False),
        donate_argnums=donate, keep_unused=True)

    _RUNNER = {
        "sharded": sharded,
        "in_names": in_names,
        "out_names": out_names,
        "zero_outs": zero_outs,
        "out_avals": out_avals,
    }
    return _RUNNER


def _run_spmd(per_core_inputs):
    """per_core_inputs: list of dicts (one per core). Returns list of dicts."""
    r = _get_runner()
    concat_in = [
        np.concatenate([np.asarray(per_core_inputs[c][k]) for c in range(N_CORES)],
                       axis=0)
        for k in r["in_names"]
    ]
    concat_zero = [np.zeros((N_CORES * z.shape[0],) + z.shape[1:], z.dtype)
                   for z in r["zero_outs"]]
    out_arrs = r["sharded"](*concat_in, *concat_zero)
    res = []
    for c in range(N_CORES):
        res.append({
            name: np.asarray(out_arrs[i]).reshape(
                (N_CORES,) + r["out_avals"][i].shape)[c]
            for i, name in enumerate(r["out_names"])
        })
    return res


def _reference_fallback(x, conv_w, conv_b, x_proj_w, dt_w, dt_b, A_logs, Ds):
    """Exact numpy computation; only used if A_logs != 0 (never in practice)."""
    x = np.asarray(x, np.float64)
    Bn, Cn = x.shape[0], x.shape[1]
    xs = np.zeros_like(x)
    xp = np.pad(x, ((0, 0), (0, 0), (2, 2), (2, 2)))
    for dy in range(5):
        for dx in range(5):
            xs += conv_w[None, :, dy, dx, None, None] * \
                xp[:, :, dy:dy + H, dx:dx + W]
    xs = xs + conv_b[None, :, None, None]
    xs = xs / (1 + np.exp(-xs))
    xs = xs.reshape(Bn, Cn, L)
    x_dbl = np.einsum("kc,bcl->bkl", x_proj_w, xs)
    dts, Bs, Cs = x_dbl[:, :DT_RANK], x_dbl[:, DT_RANK:DT_RANK + 1], \
        x_dbl[:, DT_RANK + 1:DT_RANK + 2]
    z = np.einsum("cr,brl->bcl", dt_w, dts) + dt_b[None, :, None]
    delta = np.logaddexp(0, z)
    A = -np.exp(A_logs[:, 0])
    a = np.exp(delta * A[None, :, None])
    b_el = delta * Bs * xs
    h = np.zeros((Bn, Cn))
    hs = np.zeros((Bn, Cn, L))
    for t in range(L):
        h = a[:, :, t] * h + b_el[:, :, t]
        hs[:, :, t] = h
    y = hs * Cs + xs * Ds[None, :, None]
    return y.reshape(Bn, Cn, H, W).astype(np.float32)


def kernel(x, conv_w, conv_b, x_proj_w, dt_w, dt_b, A_logs, Ds):
    x = np.ascontiguousarray(np.asarray(x, np.float32))
    A_logs = np.asarray(A_logs, np.float32)
    if np.abs(A_logs).max() > 1e-6:
        return _reference_fallback(x, conv_w, conv_b, x_proj_w, dt_w, dt_b,
                                   A_logs, np.asarray(Ds, np.float32))

    wd, xpw, dtw, lp = _prep_params(conv_w, conv_b, x_proj_w, dt_w, dt_b,
                                    A_logs, Ds)
    xflat = x.reshape(B, C, L)
    per_core = []
    for k in range(N_CORES):
        shard = xflat[B_LOC * k:B_LOC * (k + 1)].reshape(LANES, L)
        xh = shard.reshape(3, 4, 32, L).transpose(0, 2, 1, 3).reshape(96, 4, L)
        xh = np.ascontiguousarray(xh.astype(ml_dtypes.bfloat16))
        per_core.append({"xh": xh, "wd": wd, "xpw": xpw, "dtw": dtw, "lp": lp})

    res = _run_spmd(per_core)
    out = np.empty((B, C, H, W), np.float32)
    for k in range(N_CORES):
        y = np.asarray(res[k]["out"]).astype(np.float32)
        out[B_LOC * k:B_LOC * (k + 1)] = y.reshape(B_LOC, C, H, W)
    return out
